# revision 5
# baseline (speedup 1.0000x reference)
"""Multi-head attention (B=2, N=2048, D=1024, H=16, d_k=d_v=64) on 8
TRN2 NeuronCores.

Sharding: data parallel over batch (2) x tensor parallel over head
groups (4 heads per core). Each core computes its 4 heads' attention
plus the partial output projection (Wp rows for those heads); the host
sums the 4 partials per batch and adds the residual.

Per-core kernel layout choices:
  - scores are computed TRANSPOSED (S^T: keys on partitions, queries on
    free dim) so that attn@v needs no transposes: lhsT = v (natural
    [seq, d_v] layout), rhs = exp(S^T).
  - softmax skips max-subtraction (scores are ~N(0,1)*8*0.125 -> |s|<8,
    exp is safe in fp32) and the key-axis sum comes for free from a
    ones-column appended to v (M=65 matmul).
  - matmuls run in float32r (full PE rate at N>=256, ~tf32 precision);
    exp output / attn weights are bf16.
"""
import numpy as np

import concourse.bass as bass
import concourse.tile as tile
from concourse import mybir
from concourse.vector_clock import ScopedClock

f32 = mybir.dt.float32
f32r = mybir.dt.float32r
bf16 = mybir.dt.bfloat16

B, N, D = 2, 2048, 1024
H, DK = 16, 64
HPC = 4          # heads per core
GCOLS = HPC * DK  # 256 weight columns per core
NCORES = 8
P = 128
NKB = N // P      # 16 key blocks
NQB = N // 512    # 4 query blocks of 512
NDMC = D // P     # 8 d_model chunks
NSB = N // P      # 16 seq blocks for the projection

_cache = {}
_last_results = None


# ---------------------------------------------------------------------------
# Workarounds for this walrus build: max ONE sync wait per instruction.
# ---------------------------------------------------------------------------
_ws_counter = [0]


def _split_multi_waits(nc, limit=1):
    for f in nc.m.functions:
        for bb in f.blocks:
            new = []
            changed = False
            for inst in bb.instructions:
                si = inst.sync_info
                waits = list(si.on_wait) if si is not None and si.on_wait else []
                if len(waits) > limit:
                    changed = True
                    extra = waits[:-limit]
                    for i in range(0, len(extra), limit):
                        _ws_counter[0] += 1
                        nop = mybir.InstNoOp(
                            name=f"I-waitsplit-{_ws_counter[0]}", ins=[], outs=[]
                        )
                        nop.engine = inst.engine
                        nop.sync_info = mybir.SyncInfo(
                            on_wait=extra[i : i + limit], on_update=[]
                        )
                        new.append(nop)
                    si.on_wait = waits[-limit:]
                    inst.sync_info = si
                new.append(inst)
            if changed:
                bb.instructions = new


def _patched_drain_and_barrier(self, tick_clock, wait_clock):
    nc = self.nc
    drain_inst = nc.sync.drain()
    wait_clock.add_sem_waits(
        drain_inst.ins, ScopedClock({None: tick_clock.global_clock})
    )
    si = drain_inst.ins.sync_info
    if si is not None and si.on_wait is not None and len(si.on_wait) > 1:
        waits = list(si.on_wait)
        si.on_wait = waits[:1]
        drain_inst.ins.sync_info = si
        for i in range(1, len(waits)):
            extra = nc.sync.drain()
            esi = extra.ins.sync_info
            if esi is None:
                esi = mybir.SyncInfo(on_wait=[], on_update=[])
            esi.on_wait = waits[i : i + 1]
            extra.ins.sync_info = esi
    nc.all_engine_barrier()
    assert self.sems is not None
    popped = nc._tile_sem_poison_stack.pop()
    assert popped is self._sem_poison
    nc.clear_and_free_semaphores(list(self.sems.allocated().values()))
    nc.all_engine_barrier()


tile.TileContext._drain_and_barrier = _patched_drain_and_barrier


# ---------------------------------------------------------------------------
# Kernel build
# ---------------------------------------------------------------------------
def _build():
    nc = bass.Bass()
    xT = nc.dram_tensor("xT", [D, N], f32r, kind="ExternalInput")
    wq = nc.dram_tensor("wq", [D, GCOLS], f32r, kind="ExternalInput")
    wk = nc.dram_tensor("wk", [D, GCOLS], f32r, kind="ExternalInput")
    wv = nc.dram_tensor("wv", [D, GCOLS], f32r, kind="ExternalInput")
    wp = nc.dram_tensor("wp", [GCOLS, D], f32r, kind="ExternalInput")
    ones = nc.dram_tensor("ones", [1, P], f32r, kind="ExternalInput")
    pout = nc.dram_tensor("pout", [N, D], f32, kind="ExternalOutput")

    with tile.TileContext(nc) as tc:
        import contextlib

        with contextlib.ExitStack() as ctx:
            sbX = ctx.enter_context(tc.tile_pool(name="sbX", bufs=1))
            sbW = ctx.enter_context(tc.tile_pool(name="sbW", bufs=1))
            sbQK = ctx.enter_context(tc.tile_pool(name="sbQK", bufs=1))
            sbV = ctx.enter_context(tc.tile_pool(name="sbV", bufs=1))
            sbO = ctx.enter_context(tc.tile_pool(name="sbO", bufs=1))
            sbA = ctx.enter_context(tc.tile_pool(name="sbA", bufs=4))
            sbR = ctx.enter_context(tc.tile_pool(name="sbR", bufs=2))
            sbP = ctx.enter_context(tc.tile_pool(name="sbP", bufs=3))
            psA = ctx.enter_context(tc.tile_pool(name="psA", bufs=2, space="PSUM"))
            psS = ctx.enter_context(tc.tile_pool(name="psS", bufs=2, space="PSUM"))
            psO = ctx.enter_context(tc.tile_pool(name="psO", bufs=2, space="PSUM"))

            # ---- Phase A: loads -------------------------------------------
            ones_sb = sbW.tile([1, P], f32r, tag="ones")
            nc.sync.dma_start(out=ones_sb[:], in_=ones[:])

            xt = []
            for i in range(NDMC):
                t = sbX.tile([P, N], f32r, tag=f"xt{i}")
                nc.sync.dma_start(out=t[:], in_=xT[i * P : (i + 1) * P, :])
                xt.append(t)

            wq_t, wk_t, wv_t = [], [], []
            for name, dram, lst in (("wq", wq, wq_t), ("wk", wk, wk_t),
                                    ("wv", wv, wv_t)):
                for i in range(NDMC):
                    t = sbW.tile([P, GCOLS], f32r, tag=f"{name}{i}")
                    nc.sync.dma_start(out=t[:], in_=dram[i * P : (i + 1) * P, :])
                    lst.append(t)

            wp_t = []
            for g2 in range(2):
                t = sbW.tile([P, D], f32r, tag=f"wp{g2}")
                nc.sync.dma_start(out=t[:], in_=wp[g2 * P : (g2 + 1) * P, :])
                wp_t.append(t)

            # Pre-warm the exp table so the ~2.7us ACT table load overlaps
            # the projection phase.
            warm = sbR.tile([1, 2], f32, tag="warm")
            nc.scalar.activation(
                warm[:], ones_sb[0:1, 0:2], mybir.ActivationFunctionType.Exp
            )

            # ---- Phase B: q/k/v projections -------------------------------
            # qT/kT: [128 (= pair of heads x 64), N] per pair, f32r.
            qT = [sbQK.tile([P, N], f32r, tag=f"qT{g2}", name=f"qT{g2}")
                  for g2 in range(2)]
            kT = [sbQK.tile([P, N], f32r, tag=f"kT{g2}", name=f"kT{g2}")
                  for g2 in range(2)]
            # v_aug: [128 seq, kb, head-in-pair, 65] bf16 (col 64 = ones).
            vaug = [sbV.tile([P, NKB, 2, 65], bf16, tag=f"vaug{g2}",
                             name=f"vaug{g2}") for g2 in range(2)]
            for g2 in range(2):
                nc.vector.memset(vaug[g2][:, :, :, 64:65], 1.0)

            for g2 in range(2):
                for dst, w_t in ((qT, wq_t), (kT, wk_t)):
                    for qb in range(NQB):
                        pacc = psA.tile([P, 512], f32, tag="pacc")
                        for c in range(NDMC):
                            nc.tensor.matmul(
                                pacc[:],
                                w_t[c][:, g2 * P : (g2 + 1) * P],
                                xt[c][:, qb * 512 : (qb + 1) * 512],
                                start=(c == 0),
                                stop=(c == NDMC - 1),
                            )
                        nc.vector.tensor_copy(
                            dst[g2][:, qb * 512 : (qb + 1) * 512], pacc[:]
                        )
                # v for this pair: natural layout, accumulate per seq block.
                for kb in range(NKB):
                    pacc = psA.tile([P, P], f32, tag="pacc")
                    for c in range(NDMC):
                        nc.tensor.matmul(
                            pacc[:],
                            xt[c][:, kb * P : (kb + 1) * P],
                            wv_t[c][:, g2 * P : (g2 + 1) * P],
                            start=(c == 0),
                            stop=(c == NDMC - 1),
                        )
                    nc.vector.tensor_copy(vaug[g2][:, kb, :, 0:64], pacc[:])

            # ---- Phase C: attention ---------------------------------------
            outT = [sbO.tile([P, N], f32r, tag=f"outT{g2}", name=f"outT{g2}")
                    for g2 in range(2)]

            for g2 in range(2):
                for qb in range(NQB):
                    po = [psO.tile([65, 512], f32, tag="o", name=f"po{g2}_{qb}_{h}")
                          for h in range(2)]
                    for kb in range(NKB):
                        ps = [psS.tile([P, 512], f32, tag="s", name=f"ps{g2}_{qb}_{kb}_{h}")
                              for h in range(2)]
                        at = [sbA.tile([P, 512], bf16, tag="attnT",
                                        name=f"at{g2}_{qb}_{kb}_{h}")
                              for h in range(2)]
                        for h in range(2):
                            nc.tensor.matmul(
                                ps[h][:],
                                kT[g2][h * 64 : (h + 1) * 64,
                                       kb * P : (kb + 1) * P],
                                qT[g2][h * 64 : (h + 1) * 64,
                                       qb * 512 : (qb + 1) * 512],
                                start=True,
                                stop=True,
                                tile_position=(h * 64, 0),
                            )
                            nc.scalar.activation(
                                at[h][:],
                                ps[h][:],
                                mybir.ActivationFunctionType.Exp,
                                scale=0.125,
                            )
                            nc.tensor.matmul(
                                po[h][:],
                                vaug[g2][:, kb, h, :],
                                at[h][:],
                                start=(kb == 0),
                                stop=(kb == NKB - 1),
                            )
                    # normalization: recip of the sumexp row, broadcast via
                    # PE outer product, multiply into outT (f32r).
                    for h in range(2):
                        rc = sbR.tile([1, 512], f32r, tag="recip")
                        with nc.allow_low_precision(
                            reason="f32r is 4-byte; rounding only"
                        ):
                            nc.vector.reciprocal(rc[:], po[h][64:65, :])
                        pb = psA.tile([64, 512], f32, tag="pacc")
                        nc.tensor.matmul(
                            pb[:], ones_sb[0:1, 0:64], rc[:],
                            start=True, stop=True,
                        )
                        bc = sbR.tile([64, 512], f32, tag="bcast")
                        nc.vector.tensor_copy(bc[:], pb[:])
                        nc.vector.tensor_mul(
                            outT[g2][h * 64 : (h + 1) * 64,
                                     qb * 512 : (qb + 1) * 512],
                            po[h][0:64, :],
                            bc[:],
                        )

            # ---- Phase D: output projection -------------------------------
            for sb in range(NSB):
                pp = psA.tile([P, D], f32, tag="pacc")
                for half in range(2):
                    for g2 in range(2):
                        nc.tensor.matmul(
                            pp[:, half * 512 : (half + 1) * 512],
                            outT[g2][:, sb * P : (sb + 1) * P],
                            wp_t[g2][:, half * 512 : (half + 1) * 512],
                            start=(g2 == 0),
                            stop=(g2 == 1),
                        )
                ot = sbP.tile([P, D], f32, tag="pout")
                nc.vector.tensor_copy(ot[:], pp[:])
                nc.sync.dma_start(
                    out=pout[sb * P : (sb + 1) * P, :], in_=ot[:]
                )

    _split_multi_waits(nc)
    return nc


def make_in_maps(x, Wq, Wk, Wv, Wp):
    x = np.ascontiguousarray(x, dtype=np.float32)
    Wq = np.asarray(Wq, dtype=np.float32)
    Wk = np.asarray(Wk, dtype=np.float32)
    Wv = np.asarray(Wv, dtype=np.float32)
    Wp = np.asarray(Wp, dtype=np.float32)
    ones_np = np.ones((1, P), dtype=np.float32)
    in_maps = []
    for c in range(NCORES):
        b, g = divmod(c, 4)
        cs = slice(g * GCOLS, (g + 1) * GCOLS)
        in_maps.append(
            {
                "xT": np.ascontiguousarray(x[b].T),
                "wq": np.ascontiguousarray(Wq[:, cs]),
                "wk": np.ascontiguousarray(Wk[:, cs]),
                "wv": np.ascontiguousarray(Wv[:, cs]),
                "wp": np.ascontiguousarray(Wp[cs, :]),
                "ones": ones_np,
            }
        )
    return in_maps


def kernel(x, Wq, Wk, Wv, Wp):
    global _last_results
    from concourse.bass_utils import run_bass_kernel_spmd

    x = np.ascontiguousarray(x, dtype=np.float32)

    if "nc" not in _cache:
        _cache["nc"] = _build()
    nc = _cache["nc"]

    in_maps = make_in_maps(x, Wq, Wk, Wv, Wp)
    res = run_bass_kernel_spmd(nc, in_maps, core_ids=list(range(NCORES)))
    _last_results = res

    out = np.empty((B, N, D), dtype=np.float32)
    for b in range(B):
        acc = x[b].copy()
        for g in range(4):
            acc += res.results[b * 4 + g]["pout"]
        out[b] = acc
    return out
